# revision 15
# baseline (speedup 1.0000x reference)
"""Trainium2 Bass kernel for 2D erosion (3x3 sliding-window min) on
x: (8, 4, 1024, 1024) f32, borders padded with +1e9 (pad never wins).

Strategy: pure data parallel over the 32 (b, c) images -> 4 images per core.
Device compute runs in bf16 (harness gate is rel_err < 2e-2; bf16 rounding is
monotone so min commutes with it -> error <= 2^-9): halves DMA bytes and
enables the DVE 2x bf16 mode (2 elem/cycle/lane, tensor_tensor's fastest).

Work floor: a 3-tap sliding min costs 1.5 binary ops/elem per axis via
pair-sharing (s[m]=min(x[2m+1],x[2m+2]) feeds both v[2m] and v[2m+1]),
so the separable 3x3 is 3.0 ops/elem -- vs 4.0 for the naive shift chain.
Vertical sharing uses row-strided (3D) operand APs: measured on HW these
run at full 2x. Horizontal sharing needs even/odd column planes, so input
is staged column-deinterleaved, plane-major, one PAD slot per 513-wide
row (EC row = [c0,c2,..,c1022,PAD], OC row = [PAD,c1,..,c1023]); the pad
slots make every row boundary, group boundary and image edge fall out of
flat min ops with zero fixup instructions.

Layout: partition p = 32*i + j owns 32 output rows (32j..32j+31) of image
i; its input window is 34 rows (1 halo row each side, PAD rows at image
borders) staged as two 18-row half-tiles [EC plane 18x513 | OC plane
18x513] (1.125x input halo); each half-tile feeds one pair of 8-row
output chunks.

Per half-tile (chunks A, B), 6 fused vertical ops + per chunk 3
horizontal ops, all at DVE 2x:
  s_P  = min(xP[1..15:2], xP[2..16:2])   P in {EC,OC}    (8 rows, 4104)
  veP  = min(xP[0..14:2], s_P)   -> even output rows, chunk-split out
  voP  = min(s_P, xP[3..17:2])   -> odd output rows, chunk-split out
  (v per chunk: [EC_e | EC_o | OC_e | OC_o], so ECall/OCall contiguous)
  q    = min(ECall[s], OCall[s+1])   flat across both row-groups  (4103)
  o_ec = min(OCall[s], q[s])         -> even cols                 (4103)
  o_oc = min(q[s-1], ECall[s])       -> odd cols                  (4103)
Output tile [o_ec | o_oc] (2x4104, pad slots carried to DRAM and dropped
on host). Stores ride the ACT HWDGE ring; loads configurably share it.
Modeled steady state: DVE ~53us (bound), DMA ~49.6us.
"""

import numpy as np
import ml_dtypes

import concourse.bass as bass
import concourse.bacc as bacc
import concourse.mybir as mybir
from concourse.tile import TileContext
from concourse.bass_utils import run_bass_kernel_spmd

N_CORES = 8
B, C, H, W = 8, 4, 1024, 1024
IMGS = B * C // N_CORES  # images per core = 4
P = 128                  # SBUF partitions
S = W // 2 + 1           # column-plane row width incl pad slot = 513
HT = 18                  # rows per half-tile (16 + 2 halo)
HPL = HT * S             # elems per column plane per half-tile = 9234
G = 4 * S                # 4-row group block = 2052
PAD = 1.0e9
BF16 = mybir.dt.bfloat16
MIN = mybir.AluOpType.min
NP_BF16 = ml_dtypes.bfloat16

_NC_CACHE = {}

LOOP_BODY_REPS = 4  # reps unrolled inside the hardware loop body
LOAD_RING = "sync"  # loads on the SP HWDGE ring, stores on ACT: a single
                    # ring's transfers cap at roughly half the per-core DMA
                    # bandwidth on HW (measured ~94us/rep all-on-ACT vs ~53
                    # split), so loads and stores must ride separate rings


def _emit_rep(nc, pools):
    """Emit loads -> 4 chunks (shared-pair vert + fused horiz min, store)."""
    xpool, spool, vpool, qpool, opool = pools
    x, y = nc._x, nc._y

    H2 = 2 * G  # row-group pair block = 4104

    for pair in range(2):  # chunk pair = one 18-row half-tile
        # half-tile load: [EC plane 18xS | OC plane 18xS] on the SP ring.
        xt = xpool.tile([P, 2 * HPL], BF16)
        eng = nc.scalar if LOAD_RING == "scalar" else nc.sync
        eng.dma_start(
            out=xt,
            in_=bass.AP(x, pair * 2 * HPL, [[4 * HPL, P], [1, 2 * HPL]]),
        )
        xEC = xt[:, 0:HPL].rearrange("p (r w) -> p r w", w=S)  # [P, 18, S]
        xOC = xt[:, HPL : 2 * HPL].rearrange("p (r w) -> p r w", w=S)

        # vertical pass, pair-shared, fused over both chunks of the pair
        # (row-strided 3D ins; v outs split per chunk via a 2-block 3D AP)
        s = spool.tile([P, 2 * H2], BF16)  # [s_EC (8 rows) | s_OC]
        v = vpool.tile([P, 2 * 4 * G], BF16)  # per chunk: [EC_e|EC_o|OC_e|OC_o]
        v4 = v.rearrange("p (b s) -> p b s", s=4 * G)  # [P, 2 chunks, 4G]
        for pi, xP in enumerate((xEC, xOC)):
            sP = s[:, pi * H2 : (pi + 1) * H2]
            nc.vector.tensor_tensor(
                out=sP, in0=xP[:, 1:17:2, :], in1=xP[:, 2:18:2, :], op=MIN,
            )
            nc.vector.tensor_tensor(
                out=v4[:, :, 2 * pi * G : (2 * pi + 1) * G],
                in0=xP[:, 0:16:2, :], in1=sP, op=MIN,
            )
            nc.vector.tensor_tensor(
                out=v4[:, :, (2 * pi + 1) * G : (2 * pi + 2) * G],
                in0=sP, in1=xP[:, 3:18:2, :], op=MIN,
            )

        for ci in range(2):
            c = 2 * pair + ci
            vc = v[:, ci * 4 * G : (ci + 1) * 4 * G]
            # horizontal pass with pair-sharing, flat across both row-groups:
            # ECall = vc[0:2G], OCall = vc[2G:4G]; q[s] = min(v[2s], v[2s+1]).
            q = qpool.tile([P, H2], BF16)
            o = opool.tile([P, 2 * H2], BF16)  # [o_ec (2G) | o_oc (2G)]
            nc.vector.tensor_tensor(
                out=q[:, 0 : H2 - 1], in0=vc[:, 0 : H2 - 1],
                in1=vc[:, H2 + 1 : 2 * H2], op=MIN,
            )
            nc.vector.tensor_tensor(
                out=o[:, 0 : H2 - 1], in0=vc[:, H2 : 2 * H2 - 1],
                in1=q[:, 0 : H2 - 1], op=MIN,
            )
            nc.vector.tensor_tensor(
                out=o[:, H2 + 1 : 2 * H2], in0=q[:, 0 : H2 - 1],
                in1=vc[:, 1:H2], op=MIN,
            )

            # store on the ACT HWDGE ring
            nc.scalar.dma_start(
                out=y[:, c * 2 * H2 : (c + 1) * 2 * H2], in_=o
            )


def _build_nc(reps=1):
    nc = bacc.Bacc()
    nc._x = nc.dram_tensor("x", (P, 4 * HPL), BF16, kind="ExternalInput")
    nc._y = nc.dram_tensor("y", (P, 16 * G), BF16, kind="ExternalOutput")

    with TileContext(nc) as tc:
        with (
            tc.tile_pool(name="xp", bufs=3) as xpool,
            tc.tile_pool(name="sp", bufs=1) as spool,
            tc.tile_pool(name="vp", bufs=1) as vpool,
            tc.tile_pool(name="qp", bufs=1) as qpool,
            tc.tile_pool(name="op", bufs=2) as opool,
        ):
            pools = (xpool, spool, vpool, qpool, opool)
            if reps <= 48:
                for _ in range(reps):
                    _emit_rep(nc, pools)
            else:
                # timing mode: hardware loop keeps the NEFF compact so reps
                # can be large enough to swamp host/tunnel timing noise
                n_iter, rem = divmod(reps, LOOP_BODY_REPS)
                with tc.For_i(0, n_iter, 1):
                    for _ in range(LOOP_BODY_REPS):
                        _emit_rep(nc, pools)
                for _ in range(rem):
                    _emit_rep(nc, pools)

    nc.finalize()
    return nc


def _get_nc(reps=1):
    key = (reps, LOAD_RING)
    if key not in _NC_CACHE:
        _NC_CACHE[key] = _build_nc(reps)
    return _NC_CACHE[key]


def _to_bf16(x):
    """f32 -> bf16 with round-to-nearest-even (vectorized bit trick)."""
    u = np.ascontiguousarray(x, dtype=np.float32).view(np.uint32)
    r = ((u + 0x7FFF + ((u >> 16) & 1)) >> 16).astype(np.uint16)
    return r.view(NP_BF16)


def _stage_shard(shard_bf16):
    """(IMGS, H, W) bf16 -> (128, 4*HPL) staged column-plane half-tiles."""
    out = np.empty((P, 4 * HPL), dtype=NP_BF16)
    padrow = np.full((1, W), PAD, dtype=NP_BF16)
    padcol = np.full((H + 2, 1), PAD, dtype=NP_BF16)
    for i in range(IMGS):
        pi = np.concatenate([padrow, shard_bf16[i], padrow], axis=0)  # (1026, W)
        ec = np.concatenate([pi[:, 0::2], padcol], axis=1)  # (1026, S)
        oc = np.concatenate([padcol, pi[:, 1::2]], axis=1)
        dst = out[32 * i : 32 * i + 32]
        for h in range(2):
            # partition 32i+j, half h: window rows 32j+16h .. +18
            idx = (32 * np.arange(32)[:, None] + 16 * h
                   + np.arange(HT)[None, :])  # (32, 18)
            base = 2 * HPL * h
            dst[:, base : base + HPL] = ec[idx].reshape(32, HPL)
            dst[:, base + HPL : base + 2 * HPL] = oc[idx].reshape(32, HPL)
    return out


def _unstage_out(y16):
    """(n, 128, 16*G) bf16 -> (n, IMGS, H, W) dropping pad slots."""
    n = y16.shape[0]
    # [core, p, chunk, ecoc, rowgroup, rowinchunk, slot]
    y7 = y16.reshape(n, P, 4, 2, 2, 4, S)
    out = np.empty((n, P, 4, 8, W), dtype=y16.dtype)
    out[:, :, :, 0::2, 0::2] = y7[:, :, :, 0, 0, :, 0 : S - 1]  # e rows, e cols
    out[:, :, :, 1::2, 0::2] = y7[:, :, :, 0, 1, :, 0 : S - 1]  # o rows, e cols
    out[:, :, :, 0::2, 1::2] = y7[:, :, :, 1, 0, :, 1:S]        # e rows, o cols
    out[:, :, :, 1::2, 1::2] = y7[:, :, :, 1, 1, :, 1:S]        # o rows, o cols
    return out.reshape(n, IMGS, H, W)


def kernel(x: np.ndarray, _reps: int = 1):
    assert x.shape == (B, C, H, W)
    xb = _to_bf16(x).reshape(N_CORES, IMGS, H, W)
    nc = _get_nc(_reps)
    in_maps = [{"x": _stage_shard(xb[k])} for k in range(N_CORES)]
    res = run_bass_kernel_spmd(nc, in_maps, core_ids=list(range(N_CORES)))
    out16 = _unstage_out(np.stack([r["y"] for r in res.results], axis=0))
    # bf16 -> f32 upcast via bit shift
    out = (out16.view(np.uint16).astype(np.uint32) << 16).view(np.float32)
    return out.reshape(B, C, H, W)
